# revision 18
# baseline (speedup 1.0000x reference)
"""Trainium2 Bass kernel for nn_CopyStack (copy-mechanism vocab scatter).

Computes, for full inputs:
    enc   = tanh(encoder_outputs @ W_proj + b_proj)          [B,S,H]
    score = decoder_outputs @ enc^T + input_bias             [B,T,S]
    probs = softmax(score, axis=-1)                          [B,T,S]
    out[b,t,v] = sum_{s: inputs[b,s]==v} probs[b,t,s]        [B,T,V]

Sharding: 8 cores = (batch b, vocab half vh). Core c = 2*b + vh produces
out[b, :, vh*16000 : vh*16000+16000] directly in [T, V] layout.

Scatter strategy: the vocab scatter is computed as a dense matmul
out[t, v] = sum_s probs[t, s] * (ids[s] == v), made cheap by a host-side
bucketing of the 512 tokens into 8 vocab windows of 2048 columns each
(slot capacity 128 per window, sorted by id). The device permutes probs
columns into slot order with a 0/1 "Psl" matmul (K=s), then each window
needs a single K=128 matmul against a one-hot fp16 mask generated on the
DVE via is_equal vs an iota row. Duplicate ids land in distinct slots of
the same window and sum naturally in the matmul. Output is written fp16
(rel. quantization 5e-4, well under the 2e-2 gate) to halve write
traffic; the host casts to f32 and assembles.

Encoder/score matmuls run as float32r (FP22 multiplies, full PE rate);
softmax stays f32.
"""

import numpy as np

import concourse.bacc as bacc
import concourse.tile as tile
from concourse import mybir
from concourse.bass_utils import run_bass_kernel_spmd
from concourse.masks import make_identity

F32 = mybir.dt.float32
F32R = mybir.dt.float32r
F16 = mybir.dt.float16

B, S, T, H, V = 4, 512, 256, 1024, 32000
N_CORES = 8
KH = H // 128         # 8 hidden chunks
KS = S // 128         # 4 source chunks
TC2 = T // 128        # 2 target chunks
WIN = 2048            # vocab window width (fp16-exact iota range)
NW = 8                # windows per core (covers 16384 >= 16000 columns)
CAP = 128             # slot capacity per window (max observed group ~45)
VH = 16000            # vocab columns owned per core
OUTW = NW * WIN       # 16384 on-device output columns
NSL = WIN // 512      # 512-wide matmul slices per window


def build_bass():
    nc = bacc.Bacc()

    w = nc.dram_tensor("w", [H, H], F32R, kind="ExternalInput")        # W_proj
    eT = nc.dram_tensor("eT", [H, S], F32R, kind="ExternalInput")      # E[b]^T
    dT = nc.dram_tensor("dT", [H, T], F32R, kind="ExternalInput")      # D[b]^T
    bproj = nc.dram_tensor("bproj", [H], F32, kind="ExternalInput")
    sbias = nc.dram_tensor("sbias", [S], F32, kind="ExternalInput")   # input_bias[b]
    shift = nc.dram_tensor("shift", [CAP, NW], F16, kind="ExternalInput")
    slotsrc = nc.dram_tensor("slotsrc", [128, NW * CAP], F16, kind="ExternalInput")

    out16 = nc.dram_tensor("out16", [T, OUTW], F16, kind="ExternalOutput")

    with tile.TileContext(nc) as tc:
        with (
            tc.tile_pool(name="big", bufs=1) as big,
            tc.tile_pool(name="work", bufs=1) as work,
            tc.tile_pool(name="maskp", bufs=8) as maskp,
            tc.tile_pool(name="outp", bufs=6) as outp,
        ):
            # ---- loads: W on sync ring, the rest on scalar ring ----
            w_t = []
            for k in range(KH):
                wt = big.tile([128, H], F32R, tag=f"w{k}", name=f"w{k}")
                nc.sync.dma_start(wt[:], w[k * 128:(k + 1) * 128, :])
                w_t.append(wt)

            eT_t = []
            for k in range(KH):
                et = big.tile([128, S], F32R, tag=f"eT{k}", name=f"eT{k}")
                eT_t.append(et)
            nc.scalar.dma_start(eT_t[0][:], eT[0:128, :])

            bproj_sb = work.tile([128, KH], F32, tag="bproj")
            nc.scalar.dma_start(bproj_sb[:], bproj[:].rearrange("(c p) -> p c", p=128))
            sbias_sb = work.tile([128, KS], F32, tag="sbias")
            nc.scalar.dma_start(sbias_sb[:], sbias[:].rearrange("(c p) -> p c", p=128))
            shift_sb = work.tile([128, NW], F16, tag="shift")
            nc.scalar.dma_start(shift_sb[:], shift[:, :])
            slotsrc_sb = work.tile([128, NW * CAP], F16, tag="slotsrc")
            nc.scalar.dma_start(slotsrc_sb[:], slotsrc[:, :])

            for k in range(1, KH):
                nc.scalar.dma_start(eT_t[k][:], eT[k * 128:(k + 1) * 128, :])

            dT_t = []
            for m in range(KH):
                t_ = big.tile([128, T], F32R, tag=f"dT{m}", name=f"dT{m}")
                nc.scalar.dma_start(t_[:], dT[m * 128:(m + 1) * 128, :])
                dT_t.append(t_)

            ident = work.tile([128, 128], F32, tag="ident")
            make_identity(nc, ident[:])

            # ---- iota tables (gpsimd, off critical path) ----
            iota_row = work.tile([128, WIN], F16, tag="iota_row")
            nc.gpsimd.iota(iota_row[:], pattern=[[1, WIN]], base=0,
                           channel_multiplier=0,
                           allow_small_or_imprecise_dtypes=True)
            iota4 = work.tile([128, KS], F16, tag="iota4")
            nc.gpsimd.iota(iota4[:], pattern=[[128, KS]], base=0,
                           channel_multiplier=1,
                           allow_small_or_imprecise_dtypes=True)

            # ---- Psl[s, slot] = (slotsrc[slot] == s), fp16 0/1 ----
            psl = []
            for sc in range(KS):
                p_ = big.tile([128, NW * CAP], F16, tag=f"psl{sc}",
                              name=f"psl{sc}")
                nc.vector.tensor_tensor(
                    out=p_[:],
                    in0=iota4[:, sc:sc + 1].to_broadcast([128, NW * CAP]),
                    in1=slotsrc_sb[:],
                    op=mybir.AluOpType.is_equal,
                )
                psl.append(p_)

            # ---- one-hot window masks: mask_w[i, j] = (shift[i, w] == j) ----
            masks = []
            for wi in range(NW):
                m_ = maskp.tile([128, WIN], F16, tag="mask", name=f"mask{wi}")
                nc.vector.tensor_tensor(
                    out=m_[:],
                    in0=shift_sb[:, wi:wi + 1].to_broadcast([128, WIN]),
                    in1=iota_row[:],
                    op=mybir.AluOpType.is_equal,
                )
                masks.append(m_)

            encT = []
            for m in range(KH):
                et = big.tile([128, S], F32R, tag=f"encT{m}", name=f"encT{m}")
                encT.append(et)
            probsT = []
            for sc in range(KS):
                t_ = work.tile([128, T], F16, tag=f"probsT{sc}",
                               name=f"probsT{sc}")
                probsT.append(t_)
            pslotT = []

            with tc.tile_pool(name="acc8", bufs=1, space="PSUM") as acc8:
                # ---- encT[m] = tanh(W^T @ E^T + b)  [128 h', S]; single
                #      k-major pass over all 8 m-accumulators so the PE
                #      tracks the arriving W/E chunks and stays warm ----
                pm = {}
                for m in range(KH):
                    pm[m] = acc8.tile([128, S], F32, tag=f"pm{m}",
                                      name=f"pm{m}")
                for k in range(KH):
                    for m in range(KH):
                        nc.tensor.matmul(
                            pm[m][:],
                            lhsT=w_t[k][:, m * 128:(m + 1) * 128],
                            rhs=eT_t[k][:],
                            start=(k == 0), stop=(k == KH - 1),
                        )
                for m in range(KH):
                    nc.scalar.activation(
                        encT[m][:], pm[m][:],
                        mybir.ActivationFunctionType.Tanh,
                        bias=bproj_sb[:, m:m + 1], scale=1.0,
                    )

            with (
                tc.tile_pool(name="sc", bufs=2, space="PSUM") as scp,
                tc.tile_pool(name="tp", bufs=2, space="PSUM") as tpp,
            ):
                # ---- bias row [128, S]: input_bias broadcast over rows ----
                bias_row = work.tile([128, S], F32, tag="bias_row")
                for c in range(KS):
                    pt = tpp.tile([128, 128], F32, tag="tp", name=f"bt{c}")
                    nc.tensor.transpose(
                        out=pt[:],
                        in_=sbias_sb[:, c:c + 1].to_broadcast([128, 128]),
                        identity=ident[:],
                    )
                    nc.vector.tensor_copy(bias_row[:, c * 128:(c + 1) * 128],
                                          pt[:])

                # ---- scores, softmax, probs^T (fp16) per t-chunk ----
                probs_l = []
                for tc_i in range(TC2):
                    ps = scp.tile([128, S], F32, tag="ps", name=f"ps{tc_i}")
                    for m in range(KH):
                        nc.tensor.matmul(
                            ps[:],
                            lhsT=dT_t[m][:, tc_i * 128:(tc_i + 1) * 128],
                            rhs=encT[m][:],
                            start=(m == 0), stop=(m == KH - 1),
                        )
                    scoresb = work.tile([128, S], F32, tag=f"scoresb{tc_i}",
                                        name=f"scoresb{tc_i}")
                    nc.vector.tensor_tensor(
                        out=scoresb[:], in0=ps[:], in1=bias_row[:],
                        op=mybir.AluOpType.add,
                    )
                    rmax = work.tile([128, 1], F32, tag=f"rmax{tc_i}",
                                     name=f"rmax{tc_i}")
                    nc.vector.reduce_max(rmax[:], scoresb[:],
                                         axis=mybir.AxisListType.X)
                    nrmax = work.tile([128, 1], F32, tag=f"nrmax{tc_i}",
                                      name=f"nrmax{tc_i}")
                    nc.vector.tensor_scalar_mul(nrmax[:], rmax[:], -1.0)
                    ex = work.tile([128, S], F32, tag=f"ex{tc_i}",
                                   name=f"ex{tc_i}")
                    rsum = work.tile([128, 1], F32, tag=f"rsum{tc_i}",
                                     name=f"rsum{tc_i}")
                    nc.scalar.activation(
                        ex[:], scoresb[:], mybir.ActivationFunctionType.Exp,
                        bias=nrmax[:, :1], scale=1.0, accum_out=rsum[:, :1],
                    )
                    rinv = work.tile([128, 1], F32, tag=f"rinv{tc_i}",
                                     name=f"rinv{tc_i}")
                    nc.vector.reciprocal(rinv[:], rsum[:])
                    probs = work.tile([128, S], F32, tag=f"probs{tc_i}",
                                      name=f"probs{tc_i}")
                    nc.vector.tensor_scalar_mul(probs[:], ex[:], rinv[:, :1])
                    probs_l.append(probs)

                # transposes in a second pass so scores-tc1 matmuls cover
                # the softmax-tc0 vector chain instead of the PE stalling
                for tc_i in range(TC2):
                    probs = probs_l[tc_i]
                    for sc in range(KS):
                        pt = tpp.tile([128, 128], F32, tag="tp",
                                      name=f"pt{tc_i}_{sc}")
                        nc.tensor.transpose(
                            out=pt[:], in_=probs[:, sc * 128:(sc + 1) * 128],
                            identity=ident[:],
                        )
                        if sc % 2 == 0:
                            nc.vector.tensor_copy(
                                probsT[sc][:, tc_i * 128:(tc_i + 1) * 128],
                                pt[:])
                        else:
                            nc.scalar.copy(
                                probsT[sc][:, tc_i * 128:(tc_i + 1) * 128],
                                pt[:])

            # ---- scatter: out[t, w*WIN + j] = pslotT_w^T @ mask_w ----
            out_v = out16[:, :].rearrange("(c p) j -> p c j", p=128)
            with (
                tc.tile_pool(name="pj", bufs=2, space="PSUM") as pjp,
                tc.tile_pool(name="so", bufs=6, space="PSUM") as sop,
            ):
                for wi in range(NW):
                    pj = pjp.tile([128, T], F32, tag="pj", name=f"pj{wi}")
                    for sc in range(KS):
                        nc.tensor.matmul(
                            pj[:],
                            lhsT=psl[sc][:, wi * CAP:(wi + 1) * CAP],
                            rhs=probsT[sc][:],
                            start=(sc == 0), stop=(sc == KS - 1),
                        )
                    pw = work.tile([128, T], F16, tag=f"pslotT{wi}",
                                   name=f"pslotT{wi}")
                    nc.scalar.copy(pw[:], pj[:])
                    pslotT.append(pw)
                    ob = outp.tile([128, TC2 * WIN], F16, tag="ob",
                                   name=f"ob{wi}")
                    for tc_i in range(TC2):
                        for n in range(NSL):
                            po = sop.tile([128, 512], F32, tag="so",
                                          name=f"po{wi}_{tc_i}_{n}")
                            nc.tensor.matmul(
                                po[:],
                                lhsT=pslotT[wi][:, tc_i * 128:(tc_i + 1) * 128],
                                rhs=masks[wi][:, n * 512:(n + 1) * 512],
                                start=True, stop=True,
                            )
                            dst = ob[:, tc_i * WIN + n * 512:
                                     tc_i * WIN + (n + 1) * 512]
                            if (tc_i * NSL + n) % 2 == 0:
                                nc.vector.tensor_copy(dst, po[:])
                            else:
                                nc.scalar.copy(dst, po[:])
                    eng = (nc.sync, nc.scalar, nc.gpsimd)[wi % 3]
                    eng.dma_start(
                        out_v[:, :, wi * WIN:(wi + 1) * WIN],
                        ob[:].rearrange("p (c j) -> p c j", c=TC2),
                    )

    nc.finalize()
    return nc


_NC_CACHE = None


def _get_nc():
    global _NC_CACHE
    if _NC_CACHE is None:
        _NC_CACHE = build_bass()
    return _NC_CACHE


def _prep_slots(ids_b: np.ndarray, lo: int):
    """Bucket tokens with lo <= id < lo+VH into NW windows of WIN columns."""
    d = ids_b.astype(np.int64) - lo
    sel = (d >= 0) & (d < VH)
    s_idx = np.nonzero(sel)[0]
    dv = d[sel]
    wins = dv // WIN
    offs = dv % WIN
    slot_src = np.full(NW * CAP, -1.0, np.float32)
    shift = np.full((CAP, NW), -1.0, np.float32)
    for wi in range(NW):
        m = wins == wi
        cnt = int(m.sum())
        assert cnt <= CAP, f"window overflow: {cnt} > {CAP}"
        slot_src[wi * CAP:wi * CAP + cnt] = s_idx[m]
        shift[:cnt, wi] = offs[m]
    slotsrc_rep = np.ascontiguousarray(
        np.broadcast_to(slot_src.astype(np.float16)[None, :], (128, NW * CAP)))
    return slotsrc_rep, shift.astype(np.float16)


def kernel(**inputs: np.ndarray) -> np.ndarray:
    E = np.asarray(inputs["encoder_outputs"], dtype=np.float32)
    D = np.asarray(inputs["decoder_outputs"], dtype=np.float32)
    ids = np.asarray(inputs["inputs"]).astype(np.int64)
    ib = np.ascontiguousarray(np.asarray(inputs["input_bias"], dtype=np.float32))
    W = np.ascontiguousarray(np.asarray(inputs["W_proj"], dtype=np.float32))
    bp = np.ascontiguousarray(np.asarray(inputs["b_proj"], dtype=np.float32))

    nc = _get_nc()
    in_maps = []
    eT_b = [np.ascontiguousarray(E[b].T) for b in range(B)]
    dT_b = [np.ascontiguousarray(D[b].T) for b in range(B)]
    for c in range(N_CORES):
        b, vh = c // 2, c % 2
        slotsrc_rep, shift16 = _prep_slots(ids[b], vh * VH)
        in_maps.append({
            "w": W,
            "eT": eT_b[b],
            "dT": dT_b[b],
            "bproj": bp,
            "sbias": ib[b],
            "shift": shift16,
            "slotsrc": slotsrc_rep,
        })
    res = run_bass_kernel_spmd(nc, in_maps, core_ids=list(range(N_CORES)))
    out = np.empty((B, T, V), dtype=np.float32)
    for c in range(N_CORES):
        b, vh = c // 2, c % 2
        out[b, :, vh * VH:(vh + 1) * VH] = res.results[c]["out16"][:, :VH]
    return out


if __name__ == "__main__":
    nc = build_bass()
    print("built ok")
